# revision 19
# baseline (speedup 1.0000x reference)
"""Trainium2 Bass kernel for CustomLSTM (T=512, B=64, I=512, H=1024) on 8 NeuronCores.

Sharding: each core owns a 128-row slice of every gate's weight matrix
(rows [128k, 128k+128) of f/i/o/g), computes that slice of the gates, the
cell state c, and h for all 64 batch elements each timestep, and exchanges
its h-slice with the 7 peers via SBUF->SBUF remote DMA (one single-dest
broadcast instruction per peer, XOR-relative addressing).  The XOR routing
means receiver j stores core k's h-chunk at slot d = k ^ j; each core's
weight K-chunks are permuted host-side to match, so every access pattern in
the kernel is static.

The per-step matmul is fused: K = [x_t (4 chunks) | bias (1 chunk) | h (8
chunks)], M = 4 gate tiles of 128 rows, N = 64 batch.  Bias rides as K-chunk
4 with a constant e0 moving operand.  All gates use Sigmoid only (cell gate
and tanh(c) use tanh(x) = 2*sigmoid(2x)-1 with the 2x folded into weights /
activation scale) so only one ACT table set is ever needed.
"""

import os
import sys
import contextlib

import numpy as np

for _p in ("/opt/trn_rl_repo", "/opt/trn_rl_repo/concourse"):
    if _p not in sys.path:
        sys.path.insert(0, _p)

T, B, I, H = 512, 64, 512, 1024
NCORES = 8
BLK = 8          # timesteps per x-block DMA
P = 128          # partitions

# Logical->physical NC map on trn2 (driver remap).  The remote-DMA relative
# addressing XORs the slot delta with the *physical* TPB index, so the
# h-chunk a core finds in slot d is from logical core PHYS_INV[PHYS[k] ^ d].
PHYS = (0, 1, 2, 3, 6, 7, 4, 5)
PHYS_INV = tuple(PHYS.index(i) for i in range(8))


def chunk_for_slot(k, d):
    return d  # AllGather output is rank-ordered

# dtype config (fp32 first; bf16 is the perf config)
W_DT = "bfloat16"  # weights + x + h (matmul operands)


def build_nc(w_dt_name=W_DT, n_steps=T):
    import concourse.bass as bass
    import concourse.mybir as mybir

    f32 = mybir.dt.float32
    wdt = getattr(mybir.dt, "bfloat16" if w_dt_name == "bfloat16" else "float32")
    AF = mybir.ActivationFunctionType
    Alu = mybir.AluOpType

    nc = bass.Bass()
    nc.num_devices = NCORES

    wT = nc.declare_dram_parameter("wT", [P, 13 * 512], wdt, isOutput=False)
    xTr = nc.declare_dram_parameter("xTr", [n_steps, P, 256], wdt, isOutput=False)
    e0 = nc.declare_dram_parameter("e0", [P, B], wdt, isOutput=False)
    out_loc = nc.declare_dram_parameter("out_loc", [n_steps, P, B], f32, isOutput=True)
    c_out = nc.declare_dram_parameter("c_out", [P, B], f32, isOutput=True)
    ib = nc.dram_tensor("ib", [P, B], wdt)
    ob = nc.dram_tensor("ob", [8 * P, B], wdt, addr_space="Shared")

    es = contextlib.ExitStack()
    sb_wT = es.enter_context(nc.sbuf_tensor("sb_wT", [P, 13 * 512], wdt))
    sb_x = es.enter_context(nc.sbuf_tensor("sb_x", [P, 2 * BLK * 256], wdt))
    sb_e0 = es.enter_context(nc.sbuf_tensor("sb_e0", [P, B], wdt))
    sb_h = es.enter_context(nc.sbuf_tensor("sb_h", [P, 2 * 8 * B], wdt))
    sb_g = es.enter_context(nc.sbuf_tensor("sb_g", [P, 2 * 256], f32))
    sb_tc = es.enter_context(nc.sbuf_tensor("sb_tc", [P, 2 * B], f32))
    sb_c = es.enter_context(nc.sbuf_tensor("sb_c", [P, B], f32))
    sb_m1 = es.enter_context(nc.sbuf_tensor("sb_m1", [P, B], f32))
    sb_m2 = es.enter_context(nc.sbuf_tensor("sb_m2", [P, B], f32))
    sb_u = es.enter_context(nc.sbuf_tensor("sb_u", [P, B], f32))
    sb_v = es.enter_context(nc.sbuf_tensor("sb_v", [P, B], f32))
    sb_ho = es.enter_context(nc.sbuf_tensor("sb_ho", [P, 2 * B], f32))
    sb_hm = es.enter_context(nc.sbuf_tensor("sb_hm", [P, 2 * B], wdt))
    ps = [
        es.enter_context(nc.psum_tensor("ps0", [P, 512], f32)),
        es.enter_context(nc.psum_tensor("ps1", [P, 512], f32)),
    ]

    init_sem = es.enter_context(nc.semaphore("init_sem"))
    initmem = es.enter_context(nc.semaphore("initmem"))
    xdma = [es.enter_context(nc.semaphore(f"xdma{pp}")) for pp in range(2)]
    outdma = [es.enter_context(nc.semaphore(f"outdma{pp}")) for pp in range(2)]
    mm_step = es.enter_context(nc.semaphore("mm_step"))
    act_free = es.enter_context(nc.semaphore("act_free"))
    act_tc = es.enter_context(nc.semaphore("act_tc"))
    dve_c = es.enter_context(nc.semaphore("dve_c"))
    hready = es.enter_context(nc.semaphore("hready"))
    hg = [es.enter_context(nc.semaphore(f"hg{pp}")) for pp in range(2)]
    cc_sem = es.enter_context(nc.semaphore("cc_sem"))
    gp_dma = es.enter_context(nc.semaphore("gp_dma"))
    dvech = es.enter_context(nc.semaphore("dvech"))

    n_blocks = (n_steps + BLK - 1) // BLK
    mm_ctx = contextlib.ExitStack()

    def wtile(kc, g):
        return sb_wT[:, kc * 512 + g * 128 : kc * 512 + (g + 1) * 128]

    def xchunk(t, kc):
        base = (t // BLK) % 2 * (BLK * 256) + (t % BLK) * 256
        return sb_x[:, base + kc * 64 : base + (kc + 1) * 64]

    def hslot(par, d):
        return sb_h[:, par * 512 + d * B : par * 512 + (d + 1) * B]

    with nc.Block() as block:

        @block.tensor
        def _(eng):
            for t in range(n_steps):
                par = t % 2
                pst = ps[par]
                if t == 0:
                    eng.wait_ge(init_sem, 16 * 5)
                    eng.wait_ge(initmem, 2)
                if t % BLK == 0:
                    b = t // BLK
                    eng.wait_ge(xdma[b % 2], 64 * (b // 2 + 1))
                if t >= 2:
                    eng.wait_ge(act_free, t - 1)
                # x chunks + bias chunk (no h dependency)
                for kc in range(5):
                    rhs = sb_e0[:, :] if kc == 4 else xchunk(t, kc)
                    for g in range(4):
                        eng.matmul(
                            pst[:, g * 64 : (g + 1) * 64],
                            wtile(kc, g),
                            rhs,
                            start=(kc == 0 and g == 0),
                            stop=False,
                            skip_group_check=True,
                        )
                # h chunks, slot order (0 = own, then peers)
                for d in range(8):
                    if d == 0 and t >= 1:
                        eng.wait_ge(hg[t % 2], 16 * ((t + 1) // 2))
                    rhs = hslot(par, d)
                    for g in range(4):
                        ins = eng.matmul(
                            pst[:, g * 64 : (g + 1) * 64],
                            wtile(5 + d, g),
                            rhs,
                            start=False,
                            stop=(d == 7),
                            skip_group_check=True,
                        )
                ins.then_inc(mm_step, 1)

        @block.scalar
        def _(eng):
            for t in range(n_steps):
                par = t % 2
                eng.wait_ge(mm_step, t + 1)
                eng.activation(
                    sb_g[:, par * 256 : par * 256 + 256],
                    ps[par][:, 0:256],
                    AF.Sigmoid,
                ).then_inc(act_free, 1)
                eng.wait_ge(dve_c, t + 1)
                eng.activation(
                    sb_tc[:, par * B : (par + 1) * B],
                    sb_c[:, :],
                    AF.Sigmoid,
                    scale=2.0,
                ).then_inc(act_tc, 1)

        @block.vector
        def _(eng):
            for t in range(n_steps):
                par = t % 2
                q = (t + 1) % 2
                gb = par * 256
                f_ap = sb_g[:, gb + 0 : gb + 64]
                i_ap = sb_g[:, gb + 64 : gb + 128]
                o_ap = sb_g[:, gb + 128 : gb + 192]
                s_ap = sb_g[:, gb + 192 : gb + 256]
                eng.wait_ge(act_free, t + 1)
                if t >= 1:
                    eng.wait_ge(dvech, 5 * t)   # all chained ops of step t-1 done
                    eng.wait_ge(dve_c, t)       # c(t-1) written
                # cell gate: g = 2*sigmoid(2z)-1 (2z baked into weights)
                eng.tensor_mul(sb_v[:, :], f_ap, sb_c[:, :]).then_inc(dvech, 1)
                eng.tensor_scalar(
                    sb_m1[:, :], s_ap, 2.0, -1.0, Alu.mult, Alu.add
                ).then_inc(dvech, 1)
                eng.wait_ge(dvech, 5 * t + 2)
                eng.tensor_mul(sb_u[:, :], i_ap, sb_m1[:, :]).then_inc(dvech, 1)
                eng.wait_ge(dvech, 5 * t + 3)
                eng.tensor_add(sb_c[:, :], sb_v[:, :], sb_u[:, :]).then_inc(dve_c, 1)
                eng.wait_ge(act_tc, t + 1)
                if t >= 1:
                    eng.wait_ge(hready, t)      # h(t-1) done reading sb_m2
                if t >= 2:
                    eng.wait_ge(hg[(t + 1) % 2], 16 * (t // 2))
                eng.tensor_scalar(
                    sb_m2[:, :],
                    sb_tc[:, par * B : (par + 1) * B],
                    2.0,
                    -1.0,
                    Alu.mult,
                    Alu.add,
                ).then_inc(dvech, 1)
                eng.wait_ge(dvech, 5 * t + 4)
                eng.tensor_mul(
                    sb_hm[:, q * B : (q + 1) * B], o_ap, sb_m2[:, :]
                ).then_inc(hready, 1)
                eng.wait_ge(hready, t + 1)
                if t >= 2:
                    eng.wait_ge(outdma[t % 2], 16 * (t // 2))
                eng.tensor_copy(
                    sb_ho[:, q * B : (q + 1) * B], sb_hm[:, q * B : (q + 1) * B]
                ).then_inc(dvech, 1)

        @block.gpsimd
        def _(eng):
            eng.memset(sb_h[:, 0:512], 0).then_inc(initmem, 1)
            eng.memset(sb_c[:, :], 0).then_inc(initmem, 1)
            for t in range(n_steps - 1):
                q = (t + 1) % 2
                eng.wait_ge(hready, t + 1)
                eng.dma_start(ib[:, :], sb_hm[:, q * B : (q + 1) * B]).then_inc(
                    gp_dma, 16
                )
                eng.wait_ge(gp_dma, 16 * (t + 1))
                if t >= 1:
                    eng.wait_ge(hg[t % 2], 16 * ((t + 1) // 2))
                eng.collective_compute(
                    "AllGather",
                    mybir.AluOpType.bypass,
                    ins=[ib[:, :]],
                    outs=[ob[:, :]],
                    replica_groups=[list(range(NCORES))],
                ).then_inc(cc_sem, 1)
                eng.wait_ge(cc_sem, t + 1)
                eng.dma_start(
                    bass.AP(sb_h, q * 512, [[2 * 8 * B, P], [B, 8], [1, B]]),
                    bass.AP(ob, 0, [[B, P], [P * B, 8], [1, B]]),
                ).then_inc(hg[q], 16)

        @block.sync
        def _(eng):
            for j in range(4):
                eng.dma_start(
                    sb_wT[:, j * 1664 : (j + 1) * 1664],
                    wT[:, j * 1664 : (j + 1) * 1664],
                ).then_inc(init_sem, 16)
            eng.dma_start(sb_e0[:, :], e0[:, :]).then_inc(init_sem, 16)
            for b in range(min(2, n_blocks)):
                for j in range(4):
                    eng.dma_start(
                        bass.AP(
                            sb_x,
                            (b % 2) * (BLK * 256) + j * 512,
                            [[2 * BLK * 256, P], [1, 512]],
                        ),
                        bass.AP(
                            xTr,
                            b * BLK * P * 256 + j * 2 * P * 256,
                            [[256, P], [P * 256, 2], [1, 256]],
                        ),
                    ).then_inc(xdma[b % 2], 16)
            for t in range(n_steps):
                q = (t + 1) % 2
                eng.wait_ge(dvech, 5 * t + 5)
                eng.dma_start(
                    bass.AP(out_loc, t * P * B, [[B, P], [1, B]]),
                    sb_ho[:, q * B : (q + 1) * B],
                ).then_inc(outdma[t % 2], 16)
                if t % BLK == BLK - 1:
                    b = t // BLK + 2
                    if b < n_blocks:
                        eng.wait_ge(mm_step, BLK * (b - 1))
                        for j in range(4):
                            eng.dma_start(
                                bass.AP(
                                    sb_x,
                                    (b % 2) * (BLK * 256) + j * 512,
                                    [[2 * BLK * 256, P], [1, 512]],
                                ),
                                bass.AP(
                                    xTr,
                                    b * BLK * P * 256 + j * 2 * P * 256,
                                    [[256, P], [P * 256, 2], [1, 256]],
                                ),
                            ).then_inc(xdma[b % 2], 16)
            eng.wait_ge(dve_c, n_steps)
            eng.dma_start(c_out[:, :], sb_c[:, :]).then_inc(init_sem, 16)
            eng.wait_ge(outdma[0], 16 * ((n_steps + 1) // 2))
            eng.wait_ge(outdma[1], 16 * (n_steps // 2))
            eng.wait_ge(init_sem, 96)

    return nc, es, mm_ctx


def prep_inputs(x, Wf, bf, Wi, bi, Wc, bc, Wo, bo, w_dt_name=W_DT, n_steps=T):
    """Build per-core input maps (numpy only)."""
    ndt = np.float32 if w_dt_name == "float32" else None
    # gate order in M: f, i, o, g(cell).  Cell gate pre-scaled by 2.
    W4 = np.stack([Wf, Wi, Wo, 2.0 * Wc])          # [4, H, I+H]
    b4 = np.stack([bf, bi, bo, 2.0 * bc])          # [4, H]

    xr = x[:n_steps].reshape(n_steps, B, 4, 128).transpose(0, 3, 2, 1)  # [T,128,4,64]
    xTr = np.ascontiguousarray(xr).reshape(n_steps, P, 256)

    e0 = np.zeros((P, B), np.float32)
    e0[0, :] = 1.0

    def cast(a):
        if w_dt_name == "bfloat16":
            import ml_dtypes
            return a.astype(ml_dtypes.bfloat16)
        return a.astype(np.float32)

    in_maps = []
    for k in range(NCORES):
        rows = slice(k * 128, (k + 1) * 128)
        Wl = W4[:, rows, :]                        # [4, 128, 1536]
        bl = b4[:, rows]                           # [4, 128]
        wT = np.zeros((13, 128, 512), np.float32)  # [kc, kp, g*128+m]
        for g in range(4):
            gm = slice(g * 128, (g + 1) * 128)
            for kc in range(4):
                wT[kc, :, gm] = Wl[g, :, kc * 128 : (kc + 1) * 128].T
            wT[4, 0, gm] = bl[g]
            for d in range(8):
                c = chunk_for_slot(k, d)
                wT[5 + d, :, gm] = Wl[g, :, I + c * 128 : I + (c + 1) * 128].T
        wT_flat = np.ascontiguousarray(wT.transpose(1, 0, 2)).reshape(P, 13 * 512)
        in_maps.append(
            {"wT": cast(wT_flat), "xTr": cast(xTr), "e0": cast(e0)}
        )
    return in_maps


def gather_outputs(results, n_steps=T):
    outs = np.stack([r["out_loc"] for r in results])      # [8, T, 128, 64]
    out_full = np.ascontiguousarray(
        outs.transpose(1, 3, 0, 2).reshape(n_steps, B, H)
    )
    cs = np.stack([r["c_out"] for r in results])          # [8, 128, 64]
    c_t = np.ascontiguousarray(cs.transpose(2, 0, 1).reshape(B, H))
    h_t = out_full[-1].copy()
    return out_full, h_t, c_t


def kernel(x, Wf, bf, Wi, bi, Wc, bc, Wo, bo):
    os.environ.setdefault("JAX_PLATFORMS", "")
    from concourse.bass_utils import run_bass_kernel_spmd

    nc, es, mm_ctx = build_nc()
    in_maps = prep_inputs(
        np.asarray(x, np.float32),
        np.asarray(Wf, np.float32), np.asarray(bf, np.float32),
        np.asarray(Wi, np.float32), np.asarray(bi, np.float32),
        np.asarray(Wc, np.float32), np.asarray(bc, np.float32),
        np.asarray(Wo, np.float32), np.asarray(bo, np.float32),
    )
    res = run_bass_kernel_spmd(nc, in_maps, list(range(NCORES)))
    out_full, h_t, c_t = gather_outputs(res.results)
    return out_full, h_t, c_t


if __name__ == "__main__":
    nc, es, mm_ctx = build_nc()
    print("built ok")


# revision 20
# speedup vs baseline: 1.0951x; 1.0951x over previous
"""Trainium2 Bass kernel for CustomLSTM (T=512, B=64, I=512, H=1024) on 8 NeuronCores.

Sharding: each core owns a 128-row slice of every gate's weight matrix
(rows [128k, 128k+128) of f/i/o/g), computes that slice of the gates, the
cell state c, and h for all 64 batch elements each timestep, and exchanges
its h-slice with the 7 peers via SBUF->SBUF remote DMA (one single-dest
broadcast instruction per peer, XOR-relative addressing).  The XOR routing
means receiver j stores core k's h-chunk at slot d = k ^ j; each core's
weight K-chunks are permuted host-side to match, so every access pattern in
the kernel is static.

The per-step matmul is fused: K = [x_t (4 chunks) | bias (1 chunk) | h (8
chunks)], M = 4 gate tiles of 128 rows, N = 64 batch.  Bias rides as K-chunk
4 with a constant e0 moving operand.  All gates use Sigmoid only (cell gate
and tanh(c) use tanh(x) = 2*sigmoid(2x)-1 with the 2x folded into weights /
activation scale) so only one ACT table set is ever needed.
"""

import os
import sys
import contextlib

import numpy as np

for _p in ("/opt/trn_rl_repo", "/opt/trn_rl_repo/concourse"):
    if _p not in sys.path:
        sys.path.insert(0, _p)

T, B, I, H = 512, 64, 512, 1024
NCORES = 8
BLK = 8          # timesteps per x-block DMA
P = 128          # partitions

# Logical->physical NC map on trn2 (driver remap).  The remote-DMA relative
# addressing XORs the slot delta with the *physical* TPB index, so the
# h-chunk a core finds in slot d is from logical core PHYS_INV[PHYS[k] ^ d].
PHYS = (0, 1, 2, 3, 6, 7, 4, 5)
PHYS_INV = tuple(PHYS.index(i) for i in range(8))


def chunk_for_slot(k, d):
    return d  # AllGather output is rank-ordered

# dtype config (fp32 first; bf16 is the perf config)
W_DT = "bfloat16"  # weights + x + h (matmul operands)


def build_nc(w_dt_name=W_DT, n_steps=T):
    import concourse.bass as bass
    import concourse.mybir as mybir

    f32 = mybir.dt.float32
    wdt = getattr(mybir.dt, "bfloat16" if w_dt_name == "bfloat16" else "float32")
    AF = mybir.ActivationFunctionType
    Alu = mybir.AluOpType

    nc = bass.Bass()
    nc.num_devices = NCORES

    wT = nc.declare_dram_parameter("wT", [P, 13 * 512], wdt, isOutput=False)
    xTr = nc.declare_dram_parameter("xTr", [n_steps, P, 256], wdt, isOutput=False)
    e0 = nc.declare_dram_parameter("e0", [P, B], wdt, isOutput=False)
    out_loc = nc.declare_dram_parameter("out_loc", [n_steps, P, B], f32, isOutput=True)
    c_out = nc.declare_dram_parameter("c_out", [P, B], f32, isOutput=True)
    ib = nc.dram_tensor("ib", [P, B], wdt)
    ob = nc.dram_tensor("ob", [8 * P, B], wdt, addr_space="Shared")

    es = contextlib.ExitStack()
    sb_wT = es.enter_context(nc.sbuf_tensor("sb_wT", [P, 13 * 512], wdt))
    sb_x = es.enter_context(nc.sbuf_tensor("sb_x", [P, 2 * BLK * 256], wdt))
    sb_e0 = es.enter_context(nc.sbuf_tensor("sb_e0", [P, B], wdt))
    sb_h = es.enter_context(nc.sbuf_tensor("sb_h", [P, 2 * 8 * B], wdt))
    sb_g = es.enter_context(nc.sbuf_tensor("sb_g", [P, 2 * 256], f32))
    sb_tc = es.enter_context(nc.sbuf_tensor("sb_tc", [P, 2 * B], f32))
    sb_c = es.enter_context(nc.sbuf_tensor("sb_c", [P, B], f32))
    sb_m1 = es.enter_context(nc.sbuf_tensor("sb_m1", [P, B], f32))
    sb_m2 = es.enter_context(nc.sbuf_tensor("sb_m2", [P, B], f32))
    sb_u = es.enter_context(nc.sbuf_tensor("sb_u", [P, B], f32))
    sb_v = es.enter_context(nc.sbuf_tensor("sb_v", [P, B], f32))
    sb_ho = es.enter_context(nc.sbuf_tensor("sb_ho", [P, 2 * B], f32))
    sb_hm = es.enter_context(nc.sbuf_tensor("sb_hm", [P, 2 * B], wdt))
    ps = [
        es.enter_context(nc.psum_tensor("ps0", [P, 512], f32)),
        es.enter_context(nc.psum_tensor("ps1", [P, 512], f32)),
    ]

    init_sem = es.enter_context(nc.semaphore("init_sem"))
    initmem = es.enter_context(nc.semaphore("initmem"))
    xdma = [es.enter_context(nc.semaphore(f"xdma{pp}")) for pp in range(2)]
    outdma = [es.enter_context(nc.semaphore(f"outdma{pp}")) for pp in range(2)]
    mm_step = es.enter_context(nc.semaphore("mm_step"))
    act_free = es.enter_context(nc.semaphore("act_free"))
    act_tc = es.enter_context(nc.semaphore("act_tc"))
    dve_c = es.enter_context(nc.semaphore("dve_c"))
    hready = es.enter_context(nc.semaphore("hready"))
    hg = [es.enter_context(nc.semaphore(f"hg{pp}")) for pp in range(2)]
    cc_sem = es.enter_context(nc.semaphore("cc_sem"))
    gp_dma = es.enter_context(nc.semaphore("gp_dma"))
    dvech = es.enter_context(nc.semaphore("dvech"))

    n_blocks = (n_steps + BLK - 1) // BLK
    mm_ctx = contextlib.ExitStack()

    def wtile(kc, g):
        return sb_wT[:, kc * 512 + g * 128 : kc * 512 + (g + 1) * 128]

    def xchunk(t, kc):
        base = (t // BLK) % 2 * (BLK * 256) + (t % BLK) * 256
        return sb_x[:, base + kc * 64 : base + (kc + 1) * 64]

    def hslot(par, d):
        return sb_h[:, par * 512 + d * B : par * 512 + (d + 1) * B]

    with nc.Block() as block:

        @block.tensor
        def _(eng):
            for t in range(n_steps):
                par = t % 2
                pst = ps[par]
                if t == 0:
                    eng.wait_ge(init_sem, 16 * 5)
                    eng.wait_ge(initmem, 2)
                if t % BLK == 0:
                    b = t // BLK
                    eng.wait_ge(xdma[b % 2], 64 * (b // 2 + 1))
                if t >= 2:
                    eng.wait_ge(act_free, t - 1)
                # x chunks + bias chunk (no h dependency)
                for kc in range(5):
                    rhs = sb_e0[:, :] if kc == 4 else xchunk(t, kc)
                    for g in range(4):
                        eng.matmul(
                            pst[:, g * 64 : (g + 1) * 64],
                            wtile(kc, g),
                            rhs,
                            start=(kc == 0 and g == 0),
                            stop=False,
                            skip_group_check=True,
                        )
                # h chunks, slot order (0 = own, then peers)
                for d in range(8):
                    if d == 0 and t >= 1:
                        eng.wait_ge(hg[t % 2], 16 * ((t + 1) // 2))
                    rhs = hslot(par, d)
                    for g in range(4):
                        ins = eng.matmul(
                            pst[:, g * 64 : (g + 1) * 64],
                            wtile(5 + d, g),
                            rhs,
                            start=False,
                            stop=(d == 7),
                            skip_group_check=True,
                        )
                ins.then_inc(mm_step, 1)

        @block.scalar
        def _(eng):
            for t in range(n_steps):
                par = t % 2
                eng.wait_ge(mm_step, t + 1)
                eng.activation(
                    sb_g[:, par * 256 : par * 256 + 256],
                    ps[par][:, 0:256],
                    AF.Sigmoid,
                ).then_inc(act_free, 1)
                eng.wait_ge(dve_c, t + 1)
                eng.activation(
                    sb_tc[:, par * B : (par + 1) * B],
                    sb_c[:, :],
                    AF.Sigmoid,
                    scale=2.0,
                ).then_inc(act_tc, 1)

        @block.vector
        def _(eng):
            for t in range(n_steps):
                par = t % 2
                q = (t + 1) % 2
                gb = par * 256
                f_ap = sb_g[:, gb + 0 : gb + 64]
                i_ap = sb_g[:, gb + 64 : gb + 128]
                o_ap = sb_g[:, gb + 128 : gb + 192]
                s_ap = sb_g[:, gb + 192 : gb + 256]
                eng.wait_ge(act_free, t + 1)
                if t >= 1:
                    eng.wait_ge(dvech, 5 * t)   # all chained ops of step t-1 done
                    eng.wait_ge(dve_c, t)       # c(t-1) written
                # cell gate: g = 2*sigmoid(2z)-1 (2z baked into weights)
                eng.tensor_mul(sb_v[:, :], f_ap, sb_c[:, :]).then_inc(dvech, 1)
                eng.tensor_scalar(
                    sb_m1[:, :], s_ap, 2.0, -1.0, Alu.mult, Alu.add
                ).then_inc(dvech, 1)
                eng.wait_ge(dvech, 5 * t + 2)
                eng.tensor_mul(sb_u[:, :], i_ap, sb_m1[:, :]).then_inc(dvech, 1)
                eng.wait_ge(dvech, 5 * t + 3)
                eng.tensor_add(sb_c[:, :], sb_v[:, :], sb_u[:, :]).then_inc(dve_c, 1)
                eng.wait_ge(act_tc, t + 1)
                if t >= 1:
                    eng.wait_ge(hready, t)      # h(t-1) done reading sb_m2
                if t >= 2:
                    eng.wait_ge(hg[(t + 1) % 2], 16 * (t // 2))
                eng.tensor_scalar(
                    sb_m2[:, :],
                    sb_tc[:, par * B : (par + 1) * B],
                    2.0,
                    -1.0,
                    Alu.mult,
                    Alu.add,
                ).then_inc(dvech, 1)
                eng.wait_ge(dvech, 5 * t + 4)
                eng.tensor_mul(
                    sb_hm[:, q * B : (q + 1) * B], o_ap, sb_m2[:, :]
                ).then_inc(hready, 1)
                eng.wait_ge(hready, t + 1)
                if t >= 2:
                    eng.wait_ge(outdma[t % 2], 16 * (t // 2))
                eng.tensor_copy(
                    sb_ho[:, q * B : (q + 1) * B], sb_hm[:, q * B : (q + 1) * B]
                ).then_inc(dvech, 1)

        @block.gpsimd
        def _(eng):
            eng.memset(sb_h[:, 0:512], 0).then_inc(initmem, 1)
            eng.memset(sb_c[:, :], 0).then_inc(initmem, 1)
            for t in range(n_steps - 1):
                eng.wait_ge(gp_dma, 16 * (t + 1))
                if t >= 1:
                    eng.wait_ge(hg[t % 2], 16 * ((t + 1) // 2))
                eng.collective_compute(
                    "AllGather",
                    mybir.AluOpType.bypass,
                    ins=[ib[:, :]],
                    outs=[ob[:, :]],
                    replica_groups=[list(range(NCORES))],
                ).then_inc(cc_sem, 1)

        @block.sync
        def _(eng):
            for j in range(4):
                eng.dma_start(
                    sb_wT[:, j * 1664 : (j + 1) * 1664],
                    wT[:, j * 1664 : (j + 1) * 1664],
                ).then_inc(init_sem, 16)
            eng.dma_start(sb_e0[:, :], e0[:, :]).then_inc(init_sem, 16)
            for b in range(min(2, n_blocks)):
                for j in range(4):
                    eng.dma_start(
                        bass.AP(
                            sb_x,
                            (b % 2) * (BLK * 256) + j * 512,
                            [[2 * BLK * 256, P], [1, 512]],
                        ),
                        bass.AP(
                            xTr,
                            b * BLK * P * 256 + j * 2 * P * 256,
                            [[256, P], [P * 256, 2], [1, 256]],
                        ),
                    ).then_inc(xdma[b % 2], 16)
            for t in range(n_steps):
                q = (t + 1) % 2
                if t < n_steps - 1:
                    eng.wait_ge(hready, t + 1)
                    eng.dma_start(
                        ib[:, :], sb_hm[:, q * B : (q + 1) * B]
                    ).then_inc(gp_dma, 16)
                eng.wait_ge(dvech, 5 * t + 5)
                eng.dma_start(
                    bass.AP(out_loc, t * P * B, [[B, P], [1, B]]),
                    sb_ho[:, q * B : (q + 1) * B],
                ).then_inc(outdma[t % 2], 16)
                if t < n_steps - 1:
                    eng.wait_ge(cc_sem, t + 1)
                    eng.dma_start(
                        bass.AP(sb_h, q * 512, [[2 * 8 * B, P], [B, 8], [1, B]]),
                        bass.AP(ob, 0, [[B, P], [P * B, 8], [1, B]]),
                    ).then_inc(hg[q], 16)
                if t % BLK == BLK - 1:
                    b = t // BLK + 2
                    if b < n_blocks:
                        eng.wait_ge(mm_step, BLK * (b - 1))
                        for j in range(4):
                            eng.dma_start(
                                bass.AP(
                                    sb_x,
                                    (b % 2) * (BLK * 256) + j * 512,
                                    [[2 * BLK * 256, P], [1, 512]],
                                ),
                                bass.AP(
                                    xTr,
                                    b * BLK * P * 256 + j * 2 * P * 256,
                                    [[256, P], [P * 256, 2], [1, 256]],
                                ),
                            ).then_inc(xdma[b % 2], 16)
            eng.wait_ge(dve_c, n_steps)
            eng.dma_start(c_out[:, :], sb_c[:, :]).then_inc(init_sem, 16)
            eng.wait_ge(outdma[0], 16 * ((n_steps + 1) // 2))
            eng.wait_ge(outdma[1], 16 * (n_steps // 2))
            eng.wait_ge(init_sem, 96)

    return nc, es, mm_ctx


def prep_inputs(x, Wf, bf, Wi, bi, Wc, bc, Wo, bo, w_dt_name=W_DT, n_steps=T):
    """Build per-core input maps (numpy only)."""
    ndt = np.float32 if w_dt_name == "float32" else None
    # gate order in M: f, i, o, g(cell).  Cell gate pre-scaled by 2.
    W4 = np.stack([Wf, Wi, Wo, 2.0 * Wc])          # [4, H, I+H]
    b4 = np.stack([bf, bi, bo, 2.0 * bc])          # [4, H]

    xr = x[:n_steps].reshape(n_steps, B, 4, 128).transpose(0, 3, 2, 1)  # [T,128,4,64]
    xTr = np.ascontiguousarray(xr).reshape(n_steps, P, 256)

    e0 = np.zeros((P, B), np.float32)
    e0[0, :] = 1.0

    def cast(a):
        if w_dt_name == "bfloat16":
            import ml_dtypes
            return a.astype(ml_dtypes.bfloat16)
        return a.astype(np.float32)

    in_maps = []
    for k in range(NCORES):
        rows = slice(k * 128, (k + 1) * 128)
        Wl = W4[:, rows, :]                        # [4, 128, 1536]
        bl = b4[:, rows]                           # [4, 128]
        wT = np.zeros((13, 128, 512), np.float32)  # [kc, kp, g*128+m]
        for g in range(4):
            gm = slice(g * 128, (g + 1) * 128)
            for kc in range(4):
                wT[kc, :, gm] = Wl[g, :, kc * 128 : (kc + 1) * 128].T
            wT[4, 0, gm] = bl[g]
            for d in range(8):
                c = chunk_for_slot(k, d)
                wT[5 + d, :, gm] = Wl[g, :, I + c * 128 : I + (c + 1) * 128].T
        wT_flat = np.ascontiguousarray(wT.transpose(1, 0, 2)).reshape(P, 13 * 512)
        in_maps.append(
            {"wT": cast(wT_flat), "xTr": cast(xTr), "e0": cast(e0)}
        )
    return in_maps


def gather_outputs(results, n_steps=T):
    outs = np.stack([r["out_loc"] for r in results])      # [8, T, 128, 64]
    out_full = np.ascontiguousarray(
        outs.transpose(1, 3, 0, 2).reshape(n_steps, B, H)
    )
    cs = np.stack([r["c_out"] for r in results])          # [8, 128, 64]
    c_t = np.ascontiguousarray(cs.transpose(2, 0, 1).reshape(B, H))
    h_t = out_full[-1].copy()
    return out_full, h_t, c_t


def kernel(x, Wf, bf, Wi, bi, Wc, bc, Wo, bo):
    os.environ.setdefault("JAX_PLATFORMS", "")
    from concourse.bass_utils import run_bass_kernel_spmd

    nc, es, mm_ctx = build_nc()
    in_maps = prep_inputs(
        np.asarray(x, np.float32),
        np.asarray(Wf, np.float32), np.asarray(bf, np.float32),
        np.asarray(Wi, np.float32), np.asarray(bi, np.float32),
        np.asarray(Wc, np.float32), np.asarray(bc, np.float32),
        np.asarray(Wo, np.float32), np.asarray(bo, np.float32),
    )
    res = run_bass_kernel_spmd(nc, in_maps, list(range(NCORES)))
    out_full, h_t, c_t = gather_outputs(res.results)
    return out_full, h_t, c_t


if __name__ == "__main__":
    nc, es, mm_ctx = build_nc()
    print("built ok")
